# revision 3
# baseline (speedup 1.0000x reference)
"""Bass/Trainium2 kernel for nn_Rasterizer — v11 (guarded, bf16 matmul).

Baseline's proven ACT/DVE compute balance (Pool's tensor ops measured ~2.1us
per [128,128] tile — unusable), wrapped in the v2 structural shell:
  - Bass init-tail (const-ap memsets + all-engine barrier) stripped so engines
    reach kernel code at ~5.8us instead of ~6.9us.
  - input DMA issued by ACT first thing (lands ~8.0us vs 9.1us baseline);
    ACT exp-table load overlaps the DMA flight.
  - no TileContext: manual counting semaphores, distinct-slice scratch
    buffers (no engine-pipeline hazards).
  - teardown overlap: only {PE, ACT, Pool} join the final barrier; DVE's
    PSUM->SBUF copies and Sync's output DMAs run after it, overlapped with
    the other engines' NRT semaphore-reset teardown loops. The sems consumed
    after the barrier (s_mm/s_copy/s_out) are pinned at 240-242, inside
    Sync's NRT reset slice (207-255), so no other engine's teardown can zero
    them early.
"""

import numpy as np

try:
    from concourse import bacc, bass, mybir
    from concourse.bass_utils import run_bass_kernel_spmd
except ImportError:  # repo not on sys.path in a fresh grading dir
    import sys

    sys.path.insert(0, "/opt/trn_rl_repo")
    from concourse import bacc, bass, mybir
    from concourse.bass_utils import run_bass_kernel_spmd

R = 128
S = 32
SIGMA = 0.01
NCORES = 8
B_TOTAL = 16
BPC = B_TOTAL // NCORES
N_BEZ = 16
M = N_BEZ * S  # 512
KT = M // 128  # 4
NEG_INV_2SIG2 = -1.0 / (2.0 * SIGMA**2)
NCOL = BPC * KT  # 8

F32 = mybir.dt.float32
F32R = mybir.dt.float32r
BF16 = mybir.dt.bfloat16

TRACE = False
LAST_RESULTS = None
_CACHED_NC = None


def _grids():
    mesh_lr = np.linspace(-0.25 * R, R + 1.25 * R, num=R, endpoint=False)
    mesh_ud = np.linspace(-0.4 * R, R + 0.8 * R, num=R, endpoint=False)
    X = (mesh_lr / R).astype(np.float32)
    Y = (np.flip(mesh_ud) / R).astype(np.float32)
    return X, Y


def _bezier_host(cp):
    """Replicates the reference's f32 sampling math (incl. the P2-in-t^3 bug)."""
    cp = np.asarray(cp, dtype=np.float32)
    B = cp.shape[0]
    t = np.linspace(0.0, 1.0, S).astype(np.float32)[None, None, :, None]
    P0 = cp[:, :, 0][:, :, None, :]
    P1 = cp[:, :, 1][:, :, None, :]
    P2 = cp[:, :, 2][:, :, None, :]
    P3 = cp[:, :, 3][:, :, None, :]
    omt = (1.0 - t).astype(np.float32)
    samples = (
        omt**3 * P0 + 3 * t * omt**2 * P1 + 3 * omt * t**2 * P2 + t**3 * P2
    )
    deriv = (
        3 * omt**2 * (P1 - P0) + 6 * t * omt * (P2 - P1) + 3 * t**2 * (P3 - P2)
    )
    samples = samples.reshape(B, M, 2)
    deriv = deriv.reshape(B, M, 2)
    speeds = np.linalg.norm(deriv, axis=2).astype(np.float32)
    return samples, speeds


AX = float(np.float32(2.5 / 128))
BX = float(np.float32(-0.25))
AY = float(np.float32(-2.2 / 128))
BY = float(np.float32((-51.2 + 127 * 2.2) / 128))


def _strip_init_tail(nc):
    """Remove the const-ap memsets + trailing all-engine barrier from the
    Bass entry preamble (nothing here uses the const-ap tiles; all activation
    biases are explicit APs)."""
    entry = nc.main_func.blocks[0]
    insts = entry.instructions
    start = None
    for i, inst in enumerate(insts):
        if isinstance(inst, mybir.InstMemset):
            outs = inst.outs
            ref = getattr(outs[0], "memsetref", "") if outs else ""
            if ref.startswith("const-"):
                start = i
                break
    assert start is not None, "const-ap memsets not found in entry preamble"
    kinds = {type(t).__name__ for t in insts[start:]}
    assert kinds <= {"InstMemset", "InstDrain", "InstEventSemaphore"}, kinds
    del insts[start:]


def _build_program():
    nc = bacc.Bacc("TRN2", target_bir_lowering=False, debug=False)
    ET = mybir.EngineType
    AF = mybir.ActivationFunctionType
    AL = mybir.AluOpType

    inp_d = nc.dram_tensor("inp", [128, 3 * NCOL], F32, kind="ExternalInput")
    out_d = nc.dram_tensor("out", [128, BPC * 128], F32, kind="ExternalOutput")

    _strip_init_tail(nc)

    s_pre = nc.alloc_semaphore("s_pre")
    s_in = nc.alloc_semaphore("s_in")
    s_dve = nc.alloc_semaphore("s_dve")
    s_act = nc.alloc_semaphore("s_act")
    s_mm = nc.alloc_semaphore("s_mm", num=240)
    s_copy = nc.alloc_semaphore("s_copy", num=241)
    s_out = nc.alloc_semaphore("s_out", num=242)

    inp = nc.alloc_sbuf_tensor("inp_sb", [128, 3 * NCOL], F32).ap()
    nxs = inp[:, 0:NCOL]
    nys = inp[:, NCOL : 2 * NCOL]
    lnsp = inp[:, 2 * NCOL : 3 * NCOL]

    iota = nc.alloc_sbuf_tensor("iota_sb", [128, 128], F32).ap()
    zbias = nc.alloc_sbuf_tensor("zbias_sb", [128, 1], F32).ap()
    dummy = nc.alloc_sbuf_tensor("dummy_sb", [128, 1], F32).ap()
    xb = nc.alloc_sbuf_tensor("xb_sb", [128, 128], F32).ap()
    yb = nc.alloc_sbuf_tensor("yb_sb", [128, 128], F32).ap()

    # distinct slices everywhere: no same-buffer pipeline hazards
    dxx1 = nc.alloc_sbuf_tensor("dxx1", [128, 512], F32).ap()
    dya = nc.alloc_sbuf_tensor("dya", [128, 8 * 128], F32).ap()
    sqx = [nc.alloc_sbuf_tensor(f"sqx{b}", [128, 512], F32).ap() for b in range(BPC)]
    sqy = nc.alloc_sbuf_tensor("sqy", [128, 8 * 128], F32).ap()
    gx = [nc.alloc_sbuf_tensor(f"gx{b}", [128, 512], BF16).ap() for b in range(BPC)]
    gy = nc.alloc_sbuf_tensor("gy", [128, 8 * 128], BF16).ap()
    outt = nc.alloc_sbuf_tensor("outt", [128, BPC * 128], F32).ap()
    acc = [nc.alloc_psum_tensor(f"acc{b}", [128, 128], F32).ap() for b in range(BPC)]

    # ---- ACT: input DMA first, then table-load dummy (overlaps DMA flight)
    nc.scalar.dma_start(inp[:], inp_d[:]).then_inc(s_in, 16)
    nc.scalar.activation(dummy[:], dummy[:], AF.Exp, bias=zbias[:, 0:1], scale=-1.0)

    # ---- Pool: grid iota + zbias
    nc.gpsimd.iota(
        iota[:], [[1, 128]], channel_multiplier=0,
        allow_small_or_imprecise_dtypes=True,
    ).then_inc(s_pre, 1)
    nc.gpsimd.memset(zbias[:], 0.0).then_inc(s_pre, 1)

    # ---- DVE: grids
    nc.vector.wait_ge(s_pre, 1)
    nc.vector.tensor_scalar(xb[:], iota[:], AX, BX, op0=AL.mult, op1=AL.add)
    nc.vector.tensor_scalar(
        yb[:], iota[:], AY, BY, op0=AL.mult, op1=AL.add
    ).then_inc(s_pre, 1)

    # ---- DVE chains (order defines s_dve counts):
    #  1..4: ysq00..ysq03   5..7: xsq10..xsq12   8..11: ysq10..ysq13
    nc.vector.wait_ge(s_in, 16)

    def ych(b, k):
        c = b * KT + k
        sl = slice(c * 128, (c + 1) * 128)
        nc.vector.tensor_scalar_add(dya[:, sl], yb[:], nys[:, c : c + 1])
        nc.vector.tensor_mul(sqy[:, sl], dya[:, sl], dya[:, sl]).then_inc(s_dve, 1)

    def xch(k):  # batch 1 only
        c = KT + k
        sl = slice(k * 128, (k + 1) * 128)
        nc.vector.tensor_scalar_add(dxx1[:, sl], xb[:], nxs[:, c : c + 1])
        nc.vector.tensor_mul(sqx[1][:, sl], dxx1[:, sl], dxx1[:, sl]).then_inc(
            s_dve, 1
        )

    for k in range(KT):
        ych(0, k)          # s_dve 1..4
    for k in range(3):
        xch(k)             # s_dve 5..7
    for k in range(KT):
        ych(1, k)          # s_dve 8..11

    # ---- ACT: fused squares for batch-0 x side + one for batch-1 k3, exps
    # s_act counts: gx0=1 gy00=2 gy01=3 gy02=4 gy03=5 gx1=6 gy10=7.. gy13=10
    nc.scalar.wait_ge(s_in, 16)
    nc.scalar.wait_ge(s_pre, 3)
    for k in range(KT):  # batch-0 x squares, fused on ACT
        nc.scalar.activation(
            sqx[0][:, k * 128 : (k + 1) * 128], xb[:], AF.Square,
            bias=nxs[:, k : k + 1],
        )
    nc.scalar.activation(
        gx[0][:], sqx[0][:], AF.Exp, bias=zbias[:, 0:1], scale=NEG_INV_2SIG2
    ).then_inc(s_act, 1)

    def gy_exp(b, k, dve_val):
        c = b * KT + k
        sl = slice(c * 128, (c + 1) * 128)
        if dve_val is not None:
            nc.scalar.wait_ge(s_dve, dve_val)
        nc.scalar.activation(
            gy[:, sl], sqy[:, sl], AF.Exp,
            bias=lnsp[:, c : c + 1], scale=NEG_INV_2SIG2,
        ).then_inc(s_act, 1)

    gy_exp(0, 0, 1)
    gy_exp(0, 1, 2)
    gy_exp(0, 2, 3)
    gy_exp(0, 3, 4)
    # batch-1 x: k3 fused on ACT, k0-2 from DVE
    nc.scalar.activation(
        sqx[1][:, 3 * 128 : 4 * 128], xb[:], AF.Square, bias=nxs[:, KT + 3 : KT + 4]
    )
    nc.scalar.wait_ge(s_dve, 7)
    nc.scalar.activation(
        gx[1][:], sqx[1][:], AF.Exp, bias=zbias[:, 0:1], scale=NEG_INV_2SIG2
    ).then_inc(s_act, 1)
    gy_exp(1, 0, 8)
    gy_exp(1, 1, 9)
    gy_exp(1, 2, 10)
    gy_exp(1, 3, 11)

    # ---- PE: 8 matmuls; act gate values per (b, k)
    act_gate = {
        (0, 0): 2, (0, 1): 3, (0, 2): 4, (0, 3): 5,
        (1, 0): 7, (1, 1): 8, (1, 2): 9, (1, 3): 10,
    }
    for b in range(BPC):
        for k in range(KT):
            nc.tensor.wait_ge(s_act, act_gate[(b, k)])
            c = b * KT + k
            mm = nc.tensor.matmul(
                acc[b][:],
                gy[:, c * 128 : (c + 1) * 128],
                gx[b][:, k * 128 : (k + 1) * 128],
                start=(k == 0),
                stop=(k == KT - 1),
            )
            if k == KT - 1:
                mm.then_inc(s_mm, 1)

    # ---- DVE output copies (after its chains; not in barrier)
    for b in range(BPC):
        nc.vector.wait_ge(s_mm, b + 1)
        sl = slice(b * 128, (b + 1) * 128)
        nc.vector.tensor_copy(outt[:, sl], acc[b][:]).then_inc(s_copy, 1)

    # ---- output DMAs: batch-0 half issued by the (otherwise idle) Sync
    # engine as soon as its copy lands, batch-1 half by ACT right after its
    # last exp. No completion wait: NRT's teardown (per-engine sem-reset
    # loops + final ceremony) runs another ~5us after the DMAs land, and the
    # teardown's own all-engine barrier already orders every semaphore reset
    # after all engine streams end, so no explicit compute barrier is needed.
    nc.sync.wait_ge(s_copy, 1)
    nc.sync.dma_start(out_d[:, 0:128], outt[:, 0:128]).then_inc(s_out, 16)
    # batch-1 DMA split across the ACT and SP HWDGE queues: halves the
    # 128-descriptor generation time on the critical post-matmul tail
    nc.scalar.wait_ge(s_copy, 2)
    nc.scalar.dma_start(out_d[0:64, 128:256], outt[0:64, 128:256]).then_inc(s_out, 16)
    nc.sync.wait_ge(s_copy, 2)
    nc.sync.dma_start(out_d[64:128, 128:256], outt[64:128, 128:256]).then_inc(s_out, 16)
    # complete output drain on Sync: all 48 increments = all three DMAs
    # landed. Without this, PJRT can (rarely) read the output buffers before
    # the last DMA lands — observed as a large one-off correctness failure.
    nc.sync.wait_ge(s_out, 48)

    nc.compile()
    return nc


def kernel(**inputs):
    global LAST_RESULTS, _CACHED_NC
    cp = inputs["control_points"]
    samples, speeds = _bezier_host(cp)
    lns = np.log(np.maximum(speeds, 1e-30)).astype(np.float32)

    in_maps = []
    for c in range(NCORES):
        b0 = c * BPC
        nxs = -samples[b0 : b0 + BPC, :, 0].reshape(NCOL, 128).T
        nys = -samples[b0 : b0 + BPC, :, 1].reshape(NCOL, 128).T
        lc = lns[b0 : b0 + BPC].reshape(NCOL, 128).T
        inp = np.ascontiguousarray(
            np.concatenate([nxs, nys, lc], axis=1, dtype=np.float32)
        )
        in_maps.append({"inp": inp})

    if _CACHED_NC is None:
        _CACHED_NC = _build_program()
    res = run_bass_kernel_spmd(
        _CACHED_NC,
        in_maps,
        core_ids=list(range(NCORES)),
        trace=TRACE,
    )
    LAST_RESULTS = res
    out = np.concatenate(
        [r["out"].T.reshape(BPC, 128, 128).transpose(0, 2, 1) for r in res.results],
        axis=0,
    )
    return np.ascontiguousarray(out, dtype=np.float32)


# revision 4
# speedup vs baseline: 1.0245x; 1.0245x over previous
"""Bass/Trainium2 kernel for nn_Rasterizer — v12 (b0 out via SWDGE).

Baseline's proven ACT/DVE compute balance (Pool's tensor ops measured ~2.1us
per [128,128] tile — unusable), wrapped in the v2 structural shell:
  - Bass init-tail (const-ap memsets + all-engine barrier) stripped so engines
    reach kernel code at ~5.8us instead of ~6.9us.
  - input DMA issued by ACT first thing (lands ~8.0us vs 9.1us baseline);
    ACT exp-table load overlaps the DMA flight.
  - no TileContext: manual counting semaphores, distinct-slice scratch
    buffers (no engine-pipeline hazards).
  - teardown overlap: only {PE, ACT, Pool} join the final barrier; DVE's
    PSUM->SBUF copies and Sync's output DMAs run after it, overlapped with
    the other engines' NRT semaphore-reset teardown loops. The sems consumed
    after the barrier (s_mm/s_copy/s_out) are pinned at 240-242, inside
    Sync's NRT reset slice (207-255), so no other engine's teardown can zero
    them early.
"""

import numpy as np

try:
    from concourse import bacc, bass, mybir
    from concourse.bass_utils import run_bass_kernel_spmd
except ImportError:  # repo not on sys.path in a fresh grading dir
    import sys

    sys.path.insert(0, "/opt/trn_rl_repo")
    from concourse import bacc, bass, mybir
    from concourse.bass_utils import run_bass_kernel_spmd

R = 128
S = 32
SIGMA = 0.01
NCORES = 8
B_TOTAL = 16
BPC = B_TOTAL // NCORES
N_BEZ = 16
M = N_BEZ * S  # 512
KT = M // 128  # 4
NEG_INV_2SIG2 = -1.0 / (2.0 * SIGMA**2)
NCOL = BPC * KT  # 8

F32 = mybir.dt.float32
F32R = mybir.dt.float32r
BF16 = mybir.dt.bfloat16

TRACE = False
LAST_RESULTS = None
_CACHED_NC = None


def _grids():
    mesh_lr = np.linspace(-0.25 * R, R + 1.25 * R, num=R, endpoint=False)
    mesh_ud = np.linspace(-0.4 * R, R + 0.8 * R, num=R, endpoint=False)
    X = (mesh_lr / R).astype(np.float32)
    Y = (np.flip(mesh_ud) / R).astype(np.float32)
    return X, Y


def _bezier_host(cp):
    """Replicates the reference's f32 sampling math (incl. the P2-in-t^3 bug)."""
    cp = np.asarray(cp, dtype=np.float32)
    B = cp.shape[0]
    t = np.linspace(0.0, 1.0, S).astype(np.float32)[None, None, :, None]
    P0 = cp[:, :, 0][:, :, None, :]
    P1 = cp[:, :, 1][:, :, None, :]
    P2 = cp[:, :, 2][:, :, None, :]
    P3 = cp[:, :, 3][:, :, None, :]
    omt = (1.0 - t).astype(np.float32)
    samples = (
        omt**3 * P0 + 3 * t * omt**2 * P1 + 3 * omt * t**2 * P2 + t**3 * P2
    )
    deriv = (
        3 * omt**2 * (P1 - P0) + 6 * t * omt * (P2 - P1) + 3 * t**2 * (P3 - P2)
    )
    samples = samples.reshape(B, M, 2)
    deriv = deriv.reshape(B, M, 2)
    speeds = np.linalg.norm(deriv, axis=2).astype(np.float32)
    return samples, speeds


AX = float(np.float32(2.5 / 128))
BX = float(np.float32(-0.25))
AY = float(np.float32(-2.2 / 128))
BY = float(np.float32((-51.2 + 127 * 2.2) / 128))


def _strip_init_tail(nc):
    """Remove the const-ap memsets + trailing all-engine barrier from the
    Bass entry preamble (nothing here uses the const-ap tiles; all activation
    biases are explicit APs)."""
    entry = nc.main_func.blocks[0]
    insts = entry.instructions
    start = None
    for i, inst in enumerate(insts):
        if isinstance(inst, mybir.InstMemset):
            outs = inst.outs
            ref = getattr(outs[0], "memsetref", "") if outs else ""
            if ref.startswith("const-"):
                start = i
                break
    assert start is not None, "const-ap memsets not found in entry preamble"
    kinds = {type(t).__name__ for t in insts[start:]}
    assert kinds <= {"InstMemset", "InstDrain", "InstEventSemaphore"}, kinds
    del insts[start:]


def _build_program():
    nc = bacc.Bacc("TRN2", target_bir_lowering=False, debug=False)
    ET = mybir.EngineType
    AF = mybir.ActivationFunctionType
    AL = mybir.AluOpType

    inp_d = nc.dram_tensor("inp", [128, 3 * NCOL], F32, kind="ExternalInput")
    out_d = nc.dram_tensor("out", [128, BPC * 128], F32, kind="ExternalOutput")

    _strip_init_tail(nc)

    s_pre = nc.alloc_semaphore("s_pre")
    s_in = nc.alloc_semaphore("s_in")
    s_dve = nc.alloc_semaphore("s_dve")
    s_act = nc.alloc_semaphore("s_act")
    s_mm = nc.alloc_semaphore("s_mm", num=240)
    s_copy = nc.alloc_semaphore("s_copy", num=241)
    s_out = nc.alloc_semaphore("s_out", num=242)

    inp = nc.alloc_sbuf_tensor("inp_sb", [128, 3 * NCOL], F32).ap()
    nxs = inp[:, 0:NCOL]
    nys = inp[:, NCOL : 2 * NCOL]
    lnsp = inp[:, 2 * NCOL : 3 * NCOL]

    iota = nc.alloc_sbuf_tensor("iota_sb", [128, 128], F32).ap()
    zbias = nc.alloc_sbuf_tensor("zbias_sb", [128, 1], F32).ap()
    dummy = nc.alloc_sbuf_tensor("dummy_sb", [128, 1], F32).ap()
    xb = nc.alloc_sbuf_tensor("xb_sb", [128, 128], F32).ap()
    yb = nc.alloc_sbuf_tensor("yb_sb", [128, 128], F32).ap()

    # distinct slices everywhere: no same-buffer pipeline hazards
    dxx1 = nc.alloc_sbuf_tensor("dxx1", [128, 512], F32).ap()
    dya = nc.alloc_sbuf_tensor("dya", [128, 8 * 128], F32).ap()
    sqx = [nc.alloc_sbuf_tensor(f"sqx{b}", [128, 512], F32).ap() for b in range(BPC)]
    sqy = nc.alloc_sbuf_tensor("sqy", [128, 8 * 128], F32).ap()
    gx = [nc.alloc_sbuf_tensor(f"gx{b}", [128, 512], BF16).ap() for b in range(BPC)]
    gy = nc.alloc_sbuf_tensor("gy", [128, 8 * 128], BF16).ap()
    outt = nc.alloc_sbuf_tensor("outt", [128, BPC * 128], F32).ap()
    acc = [nc.alloc_psum_tensor(f"acc{b}", [128, 128], F32).ap() for b in range(BPC)]

    # ---- ACT: input DMA first, then table-load dummy (overlaps DMA flight)
    nc.scalar.dma_start(inp[:], inp_d[:]).then_inc(s_in, 16)
    nc.scalar.activation(dummy[:], dummy[:], AF.Exp, bias=zbias[:, 0:1], scale=-1.0)

    # ---- Pool: grid iota + zbias
    nc.gpsimd.iota(
        iota[:], [[1, 128]], channel_multiplier=0,
        allow_small_or_imprecise_dtypes=True,
    ).then_inc(s_pre, 1)
    nc.gpsimd.memset(zbias[:], 0.0).then_inc(s_pre, 1)

    # ---- DVE: grids
    nc.vector.wait_ge(s_pre, 1)
    nc.vector.tensor_scalar(xb[:], iota[:], AX, BX, op0=AL.mult, op1=AL.add)
    nc.vector.tensor_scalar(
        yb[:], iota[:], AY, BY, op0=AL.mult, op1=AL.add
    ).then_inc(s_pre, 1)

    # ---- DVE chains (order defines s_dve counts):
    #  1..4: ysq00..ysq03   5..7: xsq10..xsq12   8..11: ysq10..ysq13
    nc.vector.wait_ge(s_in, 16)

    def ych(b, k):
        c = b * KT + k
        sl = slice(c * 128, (c + 1) * 128)
        nc.vector.tensor_scalar_add(dya[:, sl], yb[:], nys[:, c : c + 1])
        nc.vector.tensor_mul(sqy[:, sl], dya[:, sl], dya[:, sl]).then_inc(s_dve, 1)

    def xch(k):  # batch 1 only
        c = KT + k
        sl = slice(k * 128, (k + 1) * 128)
        nc.vector.tensor_scalar_add(dxx1[:, sl], xb[:], nxs[:, c : c + 1])
        nc.vector.tensor_mul(sqx[1][:, sl], dxx1[:, sl], dxx1[:, sl]).then_inc(
            s_dve, 1
        )

    for k in range(KT):
        ych(0, k)          # s_dve 1..4
    for k in range(3):
        xch(k)             # s_dve 5..7
    for k in range(KT):
        ych(1, k)          # s_dve 8..11

    # ---- ACT: fused squares for batch-0 x side + one for batch-1 k3, exps
    # s_act counts: gx0=1 gy00=2 gy01=3 gy02=4 gy03=5 gx1=6 gy10=7.. gy13=10
    nc.scalar.wait_ge(s_in, 16)
    nc.scalar.wait_ge(s_pre, 3)
    for k in range(KT):  # batch-0 x squares, fused on ACT
        nc.scalar.activation(
            sqx[0][:, k * 128 : (k + 1) * 128], xb[:], AF.Square,
            bias=nxs[:, k : k + 1],
        )
    nc.scalar.activation(
        gx[0][:], sqx[0][:], AF.Exp, bias=zbias[:, 0:1], scale=NEG_INV_2SIG2
    ).then_inc(s_act, 1)

    def gy_exp(b, k, dve_val):
        c = b * KT + k
        sl = slice(c * 128, (c + 1) * 128)
        if dve_val is not None:
            nc.scalar.wait_ge(s_dve, dve_val)
        nc.scalar.activation(
            gy[:, sl], sqy[:, sl], AF.Exp,
            bias=lnsp[:, c : c + 1], scale=NEG_INV_2SIG2,
        ).then_inc(s_act, 1)

    gy_exp(0, 0, 1)
    gy_exp(0, 1, 2)
    gy_exp(0, 2, 3)
    gy_exp(0, 3, 4)
    # batch-1 x: k3 fused on ACT, k0-2 from DVE
    nc.scalar.activation(
        sqx[1][:, 3 * 128 : 4 * 128], xb[:], AF.Square, bias=nxs[:, KT + 3 : KT + 4]
    )
    nc.scalar.wait_ge(s_dve, 7)
    nc.scalar.activation(
        gx[1][:], sqx[1][:], AF.Exp, bias=zbias[:, 0:1], scale=NEG_INV_2SIG2
    ).then_inc(s_act, 1)
    gy_exp(1, 0, 8)
    gy_exp(1, 1, 9)
    gy_exp(1, 2, 10)
    gy_exp(1, 3, 11)

    # ---- PE: 8 matmuls; act gate values per (b, k)
    act_gate = {
        (0, 0): 2, (0, 1): 3, (0, 2): 4, (0, 3): 5,
        (1, 0): 7, (1, 1): 8, (1, 2): 9, (1, 3): 10,
    }
    for b in range(BPC):
        for k in range(KT):
            nc.tensor.wait_ge(s_act, act_gate[(b, k)])
            c = b * KT + k
            mm = nc.tensor.matmul(
                acc[b][:],
                gy[:, c * 128 : (c + 1) * 128],
                gx[b][:, k * 128 : (k + 1) * 128],
                start=(k == 0),
                stop=(k == KT - 1),
            )
            if k == KT - 1:
                mm.then_inc(s_mm, 1)

    # ---- DVE output copies (after its chains; not in barrier)
    for b in range(BPC):
        nc.vector.wait_ge(s_mm, b + 1)
        sl = slice(b * 128, (b + 1) * 128)
        nc.vector.tensor_copy(outt[:, sl], acc[b][:]).then_inc(s_copy, 1)

    # ---- output DMAs: batch-0 half issued by the (otherwise idle) Sync
    # engine as soon as its copy lands, batch-1 half by ACT right after its
    # last exp. No completion wait: NRT's teardown (per-engine sem-reset
    # loops + final ceremony) runs another ~5us after the DMAs land, and the
    # teardown's own all-engine barrier already orders every semaphore reset
    # after all engine streams end, so no explicit compute barrier is needed.
    # batch-0 output goes out on Pool's SWDGE queue: keeps the ACT/SP HWDGE
    # queues free so the batch-1 halves' flights aren't queued behind b0's
    # 128-descriptor transfer on the SP DMA path.
    nc.gpsimd.wait_ge(s_copy, 1)
    nc.gpsimd.dma_start(out_d[:, 0:128], outt[:, 0:128]).then_inc(s_out, 16)
    # batch-1 DMA split across the ACT and SP HWDGE queues: halves the
    # 128-descriptor generation time on the critical post-matmul tail
    nc.scalar.wait_ge(s_copy, 2)
    nc.scalar.dma_start(out_d[0:64, 128:256], outt[0:64, 128:256]).then_inc(s_out, 16)
    nc.sync.wait_ge(s_copy, 2)
    nc.sync.dma_start(out_d[64:128, 128:256], outt[64:128, 128:256]).then_inc(s_out, 16)
    # complete output drain on Sync: all 48 increments = all three DMAs
    # landed. Without this, PJRT can (rarely) read the output buffers before
    # the last DMA lands — observed as a large one-off correctness failure.
    nc.sync.wait_ge(s_out, 48)

    nc.compile()
    return nc


def kernel(**inputs):
    global LAST_RESULTS, _CACHED_NC
    cp = inputs["control_points"]
    samples, speeds = _bezier_host(cp)
    lns = np.log(np.maximum(speeds, 1e-30)).astype(np.float32)

    in_maps = []
    for c in range(NCORES):
        b0 = c * BPC
        nxs = -samples[b0 : b0 + BPC, :, 0].reshape(NCOL, 128).T
        nys = -samples[b0 : b0 + BPC, :, 1].reshape(NCOL, 128).T
        lc = lns[b0 : b0 + BPC].reshape(NCOL, 128).T
        inp = np.ascontiguousarray(
            np.concatenate([nxs, nys, lc], axis=1, dtype=np.float32)
        )
        in_maps.append({"inp": inp})

    if _CACHED_NC is None:
        _CACHED_NC = _build_program()
    res = run_bass_kernel_spmd(
        _CACHED_NC,
        in_maps,
        core_ids=list(range(NCORES)),
        trace=TRACE,
    )
    LAST_RESULTS = res
    out = np.concatenate(
        [r["out"].T.reshape(BPC, 128, 128).transpose(0, 2, 1) for r in res.results],
        axis=0,
    )
    return np.ascontiguousarray(out, dtype=np.float32)
